# revision 1
# baseline (speedup 1.0000x reference)
"""Corner-pooling module kernel for 8 Trainium2 NeuronCores.

Reference computation (NCHW, fp32):
    p1 = relu(bn(conv3x3(x, w_p1)))          # 256 -> 128 ch
    p2 = relu(bn(conv3x3(x, w_p2)))          # 256 -> 128 ch
    cp1 = cummax(p1, axis=H, reverse=True)
    cp2 = cummax(p2, axis=W, reverse=True)
    r  = relu(bn(conv3x3(cp1+cp2, w_c1)) + bn(conv1x1(x, w_c2)))
    out = relu(bn(conv3x3(r, w_p3)))

Sharding: 8 cores = 4 samples x 2 H-halves (core 2b: rows 0..63 of
sample b, core 2b+1: rows 64..127).  All BN scales are folded into the
conv weights host-side (per-out-channel bias remains), so every conv is
matmul-accumulate + one ACT relu-with-bias epilogue.  Convs run as
flat-chunked matmuls over a W-padded (130-wide) layout with channels on
partitions, in float32r (full-rate PE, ~2.4e-4 eps).

Schedule (the point of this version): conv p1 runs FIRST in reverse
chunk order so the 63-step reverse H-cummax chain interleaves under it
on DVE; conv p2 runs second with its first+last chunks prioritized so
the 8-row pairwise AllReduce (the only cross-core dependency) plus all
cummax fixups overlap p2's remaining matmuls.  The W-cummax runs as 4
flat multi-row reversed scans using a (max,mult) mask-reset trick, and
the colmax-broadcast fixup / s=cp1+cp2 add / pad zeroing are split into
4 row pieces (fixups on DVE, adds on gpsimd) so conv c1 starts the
moment p2's matmuls end.  x streams highest-rows-first so conv p1's
first (reverse) chunk starts ~4us in, and a few dummy matmuls on the
wc2 tile pre-warm the PE clock gate during the initial DMA.
"""

import sys

sys.path.insert(0, "/opt/trn_rl_repo")

import numpy as np

import bass_rust
import concourse.bass as bass
import concourse.mybir as mybir
from concourse.bass_utils import run_bass_kernel_spmd
from concourse.tile import TileContext

F32 = mybir.dt.float32
F32R = mybir.dt.float32r
EPS = 1e-5

B, CIN, H, W = 4, 256, 128, 128
MID = 128
P = 128
WP = W + 2          # padded width
HS = 66             # x-slab rows: own 64 + 1 halo above + 1 below
SS = 68             # s-slab rows: own 64 + 2 halo above + 2 below
OH = 64             # own rows per core
TAPS = [(ky, kx) for ky in (-1, 0, 1) for kx in (-1, 0, 1)]
NPC = 4             # fixup/add/scan pieces
PCR = OH // NPC     # rows per piece


def _r12(x):
    """Round fp32 to float32r precision (11 explicit mantissa bits, RNE)."""
    u = np.ascontiguousarray(x, dtype=np.float32).view(np.uint32)
    r = (u + 0x7FF + ((u >> 12) & 1)) & 0xFFFFF000
    return r.view(np.float32)


def _fix_multiwaits(nc):
    """walrus in this container accepts at most ONE sem wait per
    instruction; split extras onto same-engine nops placed just before."""

    def steal_nop(eng):
        bi = nc.engines[eng].nop()
        ins = bi.ins
        cur = nc.cur_bb.bb
        lst = cur.instructions
        assert lst[-1] is ins or lst[-1].name == ins.name
        cur.instructions = lst[:-1]
        return ins

    for fn in nc.m.functions:
        for bb in fn.blocks:
            out = []
            changed = False
            for inst in bb.instructions:
                si = inst.sync_info
                waits = list(si.on_wait) if si is not None and si.on_wait else []
                if len(waits) > 1:
                    for wv in waits[:-1]:
                        nop = steal_nop(inst.engine)
                        nop.sync_info = bass_rust.SyncInfo(on_wait=[wv], on_update=[])
                        out.append(nop)
                    inst.sync_info = bass_rust.SyncInfo(
                        on_wait=[waits[-1]], on_update=list(si.on_update or [])
                    )
                    changed = True
                out.append(inst)
            if changed:
                bb.instructions = out
    return nc


def _emit_conv(nc, psum, rhs_flats, rhs_sizes, terms, nt, nchunks, epilogue,
               order=None):
    """Flat-chunked conv: for each output chunk accumulate all terms'
    matmuls in one PSUM tile, then run epilogue(chunk_idx, psum_ap).

    terms: list of (rhs_idx, lhsT_ap, off) where the term reads
    rhs_flats[rhs_idx][:, q+off : q+off+nt] for out positions [q, q+nt).
    Edge terms are trimmed by one element at the slab ends.
    order: chunk iteration order (default ascending).
    """
    for ci in (range(nchunks) if order is None else order):
        q = ci * nt
        full, part = [], []
        for rhs_idx, lhsT, off in terms:
            o = q + off
            lo = max(0, -o)
            hi = min(nt, rhs_sizes[rhs_idx] - o)
            # fp32r matmul dst needs even offset+length; the extra trimmed
            # elements only ever read zero pad positions
            lo += lo & 1
            hi -= (hi - lo) & 1
            (full if (lo == 0 and hi == nt) else part).append(
                (rhs_idx, lhsT, o, lo, hi)
            )
        seq = [full[0]] + part + full[1:]
        pt = psum.tile([P, 512], F32, tag="ps")
        n = len(seq)
        for i, (rhs_idx, lhsT, o, lo, hi) in enumerate(seq):
            nc.tensor.matmul(
                pt[:, lo:hi],
                lhsT,
                rhs_flats[rhs_idx][:, o + lo : o + hi],
                start=(i == 0),
                stop=(i == n - 1),
            )
        epilogue(ci, pt[:, :nt])


def build_nc():
    nc = bass.Bass()

    xs = nc.dram_tensor("xs", [2, P, HS, WP], F32R, kind="ExternalInput")
    wp1 = nc.dram_tensor("wp1", [P, 2, 9, P], F32R, kind="ExternalInput")
    wp2 = nc.dram_tensor("wp2", [P, 2, 9, P], F32R, kind="ExternalInput")
    wc1 = nc.dram_tensor("wc1", [P, 2, 9, P], F32R, kind="ExternalInput")
    wc2 = nc.dram_tensor("wc2", [P, 2, 2, P], F32R, kind="ExternalInput")
    wp3 = nc.dram_tensor("wp3", [P, 2, 2, 9, P], F32R, kind="ExternalInput")
    bm = nc.dram_tensor("bm", [P, 9], F32, kind="ExternalInput")
    o = nc.dram_tensor("o", [2, P, OH, W], F32, kind="ExternalOutput")

    groups = [[0, 1], [2, 3], [4, 5], [6, 7]]

    with TileContext(nc) as tc:
        with (
            tc.tile_pool(name="const", bufs=1) as cpool,
            tc.tile_pool(name="wt", bufs=2) as wpool,
            tc.tile_pool(name="psum", bufs=6, space="PSUM") as psum,
            tc.tile_pool(name="scratch", bufs=1) as spool_s,
            tc.tile_pool(name="dram", bufs=1, space="DRAM") as dpool,
            tc.tile_pool(name="sbig", bufs=1) as spool,
            tc.tile_pool(name="x", bufs=1) as xpool,
        ):
            # ---- constants: one tiny DMA ----
            bmt = cpool.tile([P, 9], F32)
            nc.sync.dma_start(bmt[:], bm[:])
            bt = bmt[:, 0:6]
            m_top = bmt[:, 6:7]
            m_bot = bmt[:, 7:8]
            coloff = bmt[:, 8:9]
            wc2t = cpool.tile([P, 2, 2, P], F32R)

            # ---- weights: slot order wp1,wp2 (so wc1/wp3a later reuse the
            # slot freed at p1/p2 end) but wp2's DMA issues first — the two
            # early p2 chunks need it before conv p1 starts ----
            wp1t = wpool.tile([P, 2, 9, P], F32R, tag="w")
            wp2t = wpool.tile([P, 2, 9, P], F32R, tag="w")
            # two pieces: the pre-warm matmuls only need the first slab, so
            # they start after ~half the transfer
            nc.sync.dma_start(wp2t[:, 0], wp2[:, 0])
            nc.sync.dma_start(wp2t[:, 1], wp2[:, 1])

            # ---- PE pre-warm: dummy matmuls on wp2 release the HAM clock
            # gate and bridge seamlessly into the first real chunk ----
            wp2fl = wp2t[:].rearrange("p a b c -> p (a b c)")
            ptw = psum.tile([P, 512], F32, tag="ps")
            for _ in range(3):
                nc.tensor.matmul(
                    ptw[:, 0:512], wp2fl[:, 0:P], wp2fl[:, 0:512],
                    start=True, stop=True,
                )

            def wtile(src):
                t = wpool.tile([P, 2, 9, P], F32R, tag="w")
                nc.sync.dma_start(t[:], src[:])
                return t

            # ---- x slabs, highest rows first (conv p1 runs reverse); the
            # rows-0 piece comes third so p2's chunk-0 (exchange rows 0,1)
            # can run early too ----
            xt = [xpool.tile([P, HS, WP], F32R, name=f"x{s}", tag=f"x{s}") for s in range(2)]
            piece_order = [HS - 6, 0, HS - 12] + list(range(HS - 18, 0, -6))
            for k, r0 in enumerate(piece_order):
                for s in range(2):
                    nc.sync.dma_start(xt[s][:, r0:r0 + 6, :], xs[s, :, r0:r0 + 6, :])
                if k == 1:
                    nc.sync.dma_start(wp1t[:], wp1[:])
                elif k == 2:
                    nc.sync.dma_start(wc2t[:], wc2[:])
            xf = [t[:].rearrange("p h w -> p (h w)") for t in xt]
            xsz = HS * WP

            # ---- s slab (conv p1 writes rows 2..65; becomes cp1, then s) ----
            st = spool.tile([P, SS, WP], F32R)
            sf = st[:].rearrange("p h w -> p (h w)")

            NT_A, NCH_A = 416, (OH * WP) // 416

            with tc.tile_pool(name="p2", bufs=1) as p2pool:
                p2t = p2pool.tile([P, OH, WP], F32R)
                p2f = p2t[:].rearrange("p h w -> p (h w)")
                # scan reset mask: 1 everywhere, 0 at the two pad cols of
                # each row (values are post-relu >= 0 so a 0-reset == -inf)
                mk = p2pool.tile([P, PCR, WP], F32)
                nc.vector.memset(mk[:], 1.0)
                nc.vector.tensor_scalar_mul(mk[:, :, 0:1], mk[:, :, 0:1], 0.0)
                nc.vector.tensor_scalar_mul(
                    mk[:, :, WP - 1 : WP], mk[:, :, WP - 1 : WP], 0.0
                )
                mkf = mk[:].rearrange("p h w -> p (h w)")

                def conv_branch(wt_tile, bias_col, out_flat, out_base, order=None):
                    terms = []
                    for s in range(2):
                        for t_i, (ky, kx) in enumerate(TAPS):
                            terms.append(
                                (s, wt_tile[:, s, t_i, :], (1 + ky) * WP + kx)
                            )

                    def epi(ci, pap):
                        ncols = pap.shape[-1]
                        nc.scalar.activation(
                            out_flat[:, out_base + ci * NT_A : out_base + ci * NT_A + ncols],
                            pap,
                            mybir.ActivationFunctionType.Relu,
                            bias=bias_col,
                        )

                    _emit_conv(nc, psum, xf, [xsz, xsz], terms, NT_A, NCH_A, epi,
                               order=order)

                # p2's exchange chunks (rows 62,63 and 0,1) run before conv
                # p1 so the collective can fire the moment the chain ends
                conv_branch(wp2t, bt[:, 1:2], p2f, 0, order=[NCH_A - 1, 0])
                # W-cummax of the 4 exchange rows (in place, reversed)
                for h in (OH - 2, OH - 1, 0, 1):
                    row = p2t[:, h, 1 : WP - 1]
                    nc.vector.tensor_tensor_scan(
                        row[:, ::-1],
                        row[:, ::-1],
                        row[:, ::-1],
                        -1e30,
                        mybir.AluOpType.max,
                        mybir.AluOpType.bypass,
                    )

                # conv p1 -> s rows 2..65, reverse chunk order
                conv_branch(wp1t, bt[:, 0:1], sf, 2 * WP,
                            order=range(NCH_A - 1, -1, -1))
                # reverse cummax over H (rows 64 -> 2), in place; chain op h
                # only needs the conv chunks covering rows h,h+1, so under
                # reverse chunk order this interleaves with conv p1 on DVE
                for h in range(OH, 1, -1):
                    nc.vector.tensor_tensor(
                        st[:, h, :],
                        st[:, h, :],
                        st[:, h + 1, :],
                        mybir.AluOpType.max,
                    )

                # ---- pairwise exchange ----
                # C slots: 0,1 cp1local first2 rows (*mb); 2,3 cp1local last2
                # (*mt); 4,5 cp2 first2 (*mb); 6,7 cp2 last2 (*mt).
                # bf16 payload: halves the collective time; only halo rows
                # and the colmax are affected (~0.4% rounding, tol is 2e-2).
                BF16 = mybir.dt.bfloat16
                ct = spool_s.tile([P, 8, WP], BF16, tag="exch")
                for k, (src, m) in enumerate(
                    [
                        (st[:, 2, :], m_bot),
                        (st[:, 3, :], m_bot),
                        (st[:, 2 + OH - 2, :], m_top),
                        (st[:, 2 + OH - 1, :], m_top),
                        (p2t[:, 0, :], m_bot),
                        (p2t[:, 1, :], m_bot),
                        (p2t[:, OH - 2, :], m_top),
                        (p2t[:, OH - 1, :], m_top),
                    ]
                ):
                    nc.vector.tensor_scalar_mul(ct[:, k, :], src, m)
                cc_in = dpool.tile([P, 8, WP], BF16)
                cc_out = dpool.tile([P, 8, WP], BF16)
                nc.sync.dma_start(cc_in[:], ct[:])
                nc.gpsimd.collective_compute(
                    "AllReduce",
                    mybir.AluOpType.add,
                    replica_groups=groups,
                    ins=[cc_in[:]],
                    outs=[cc_out[:]],
                )
                rt = spool_s.tile([P, 8, WP], BF16, tag="exch")
                nc.sync.dma_start(rt[:], cc_out[:])

                # u = R[0] + coloff (top cores: partner colmax; bottom: -inf)
                u = spool_s.tile([P, WP], F32R, tag="u")
                nc.vector.tensor_scalar_add(u[:], rt[:, 0, :], coloff)

                # conv p2's remaining chunks run while the collective is in
                # flight and the pieces below drain
                conv_branch(wp2t, bt[:, 1:2], p2f, 0,
                            order=range(1, NCH_A - 1))

                # per 16-row piece: bulk W-cummax (flat reversed scan with
                # mask-mult state reset at the pad cols; re-scanning the
                # exchange rows is idempotent), cp1 colmax fixup, s=cp1+cp2,
                # pad-col zeroing — piece 0 first so conv c1's first chunks
                # are unblocked before conv p2 even finishes.
                cm = spool_s.tile([P, WP], F32R, tag="cm")
                h0 = spool_s.tile([P, WP], F32R, tag="h0")
                h1 = spool_s.tile([P, WP], F32R, tag="h1")
                for pc in range(NPC):
                    seg = p2f[:, pc * PCR * WP : (pc + 1) * PCR * WP]
                    nc.vector.tensor_tensor_scan(
                        seg[:, ::-1],
                        seg[:, ::-1],
                        mkf[:, ::-1],
                        0.0,
                        mybir.AluOpType.max,
                        mybir.AluOpType.mult,
                    )
                    r0 = 2 + pc * PCR
                    sp = st[:, r0 : r0 + PCR, :]
                    nc.vector.tensor_tensor(
                        sp,
                        sp,
                        u[:, None, :].to_broadcast((P, PCR, WP)),
                        mybir.AluOpType.max,
                    )
                    if pc == 0:
                        # own colmax (= fixed cp1 row 0), snapshotted before
                        # the add overwrites st row 2
                        nc.vector.tensor_copy(cm[:], st[:, 2, :])
                    nc.vector.tensor_tensor(
                        sp,
                        sp,
                        p2t[:, pc * PCR : pc * PCR + PCR, :],
                        mybir.AluOpType.add,
                    )
                    nc.vector.tensor_scalar_mul(
                        st[:, r0 : r0 + PCR, 0:1], st[:, r0 : r0 + PCR, 0:1], 0.0
                    )
                    nc.vector.tensor_scalar_mul(
                        st[:, r0 : r0 + PCR, WP - 1 : WP],
                        st[:, r0 : r0 + PCR, WP - 1 : WP],
                        0.0,
                    )
                    if pc == 0:
                        # halo rows right after piece 0 (conv c1 chunk 0
                        # needs them).  above halo (bottom cores):
                        # max(partner cp1local last2, own colmax) + partner
                        # cp2 last2, *mb
                        for j, dst_row in ((0, 0), (1, 1)):
                            nc.vector.tensor_tensor(
                                h0[:], rt[:, 2 + j, :], cm[:], mybir.AluOpType.max
                            )
                            nc.vector.tensor_tensor(
                                h0[:], h0[:], rt[:, 6 + j, :], mybir.AluOpType.add
                            )
                            nc.vector.tensor_scalar_mul(
                                st[:, dst_row, :], h0[:], m_bot
                            )
                        # below halo (top cores): partner cp1local first2 +
                        # cp2 first2
                        for j, dst_row in ((0, SS - 2), (1, SS - 1)):
                            nc.vector.tensor_tensor(
                                h1[:], rt[:, 0 + j, :], rt[:, 4 + j, :],
                                mybir.AluOpType.add,
                            )
                            nc.vector.tensor_scalar_mul(
                                st[:, dst_row, :], h1[:], m_top
                            )
                        # zero pad cols of the 4 halo rows (in-place *0:
                        # memset may not produce fp32r-matmul operands under
                        # this walrus)
                        for rr in (0, SS - 2):
                            nc.vector.tensor_scalar_mul(
                                st[:, rr : rr + 2, 0:1],
                                st[:, rr : rr + 2, 0:1],
                                0.0,
                            )
                            nc.vector.tensor_scalar_mul(
                                st[:, rr : rr + 2, WP - 1 : WP],
                                st[:, rr : rr + 2, WP - 1 : WP],
                                0.0,
                            )

            # ---- conv c1 + c2 -> r_ext ----
            wc1t = wtile(wc1)
            NT_C, NCH_C = 390, ((OH + 2) * WP) // 390
            ssz = SS * WP
            with tc.tile_pool(name="r", bufs=1) as rpool:
                rt2 = [rpool.tile([P, HS, WP], F32R, name=f"r{i}", tag=f"r{i}") for i in range(2)]
                rf = [t[:].rearrange("p h w -> p (h w)") for t in rt2]
                for oh_half in range(2):
                    terms = []
                    for t_i, (ky, kx) in enumerate(TAPS):
                        terms.append((0, wc1t[:, oh_half, t_i, :], (1 + ky) * WP + kx))
                    for s in range(2):
                        terms.append((1 + s, wc2t[:, s, oh_half, :], 0))

                    def epi(ci, pap, oh_half=oh_half):
                        ncols = pap.shape[-1]
                        nc.scalar.activation(
                            rf[oh_half][:, ci * NT_C : ci * NT_C + ncols],
                            pap,
                            mybir.ActivationFunctionType.Relu,
                            bias=bt[:, 2 + oh_half : 3 + oh_half],
                        )

                    _emit_conv(
                        nc, psum, [sf, xf[0], xf[1]], [ssz, xsz, xsz],
                        terms, NT_C, NCH_C, epi,
                    )
                    # mask invalid halo rows, zero pad cols (per half so the
                    # ops overlap the other half's matmuls)
                    t = rt2[oh_half]
                    nc.vector.tensor_scalar_mul(t[:, 0, :], t[:, 0, :], m_bot)
                    nc.vector.tensor_scalar_mul(
                        t[:, HS - 1, :], t[:, HS - 1, :], m_top
                    )
                    nc.vector.tensor_scalar_mul(t[:, :, 0:1], t[:, :, 0:1], 0.0)
                    nc.vector.tensor_scalar_mul(
                        t[:, :, WP - 1 : WP], t[:, :, WP - 1 : WP], 0.0
                    )

                # ---- conv p3 -> per-half staged rows -> out ----
                # x is no longer needed; its SBUF is reused for the two
                # output stages.  NT 416 (vs 260) cuts matmul count 1152->720
                # and the 8-row output DMAs cut issue count 64->16.
                wp3a = wpool.tile([P, 2, 9, P], F32R, tag="w")
                nc.sync.dma_start(wp3a[:], wp3[:, 0])
                wp3b = wpool.tile([P, 2, 9, P], F32R, tag="w")
                nc.sync.dma_start(wp3b[:], wp3[:, 1])
                NT_O, NCH_O = 416, (OH * WP) // 416
                rsz = HS * WP
                if True:
                    for oh_half, wtile_ in ((0, wp3a), (1, wp3b)):
                        # stage reuses the x slab's SBUF slot (same tag);
                        # the alloc waits for conv c1's last x access
                        stg = xpool.tile([P, OH, WP], F32, tag=f"x{oh_half}")
                        sgf = stg[:].rearrange("p h w -> p (h w)")
                        terms = []
                        for s in range(2):
                            for t_i, (ky, kx) in enumerate(TAPS):
                                terms.append(
                                    (s, wtile_[:, s, t_i, :], (1 + ky) * WP + kx)
                                )

                        def epi(ci, pap, sgf=sgf, oh_half=oh_half):
                            ncols = pap.shape[-1]
                            nc.scalar.activation(
                                sgf[:, ci * NT_O : ci * NT_O + ncols],
                                pap,
                                mybir.ActivationFunctionType.Relu,
                                bias=bt[:, 4 + oh_half : 5 + oh_half],
                            )

                        _emit_conv(nc, psum, rf, [rsz, rsz], terms, NT_O,
                                   NCH_O, epi)
                        for r0 in range(0, OH, 4):
                            nc.sync.dma_start(
                                o[oh_half, :, r0 : r0 + 4, :],
                                stg[:, r0 : r0 + 4, 1 : WP - 1],
                            )

    _fix_multiwaits(nc)
    return nc


_NC = None


def _get_nc():
    global _NC
    if _NC is None:
        _NC = build_nc()
    return _NC


def _fold_bn(w, g, b, m, v):
    s = (g / np.sqrt(v + EPS)).astype(np.float32)
    t = (b - m * s).astype(np.float32)
    return w * s[:, None, None, None], t


def kernel(**inputs):
    x = np.asarray(inputs["x"], np.float32)

    w_p1, t_p1 = _fold_bn(
        np.asarray(inputs["w_p1"], np.float32),
        inputs["g_p1"], inputs["b_p1"], inputs["m_p1"], inputs["v_p1"],
    )
    w_p2, t_p2 = _fold_bn(
        np.asarray(inputs["w_p2"], np.float32),
        inputs["g_p2"], inputs["b_p2"], inputs["m_p2"], inputs["v_p2"],
    )
    w_c1, t_c1 = _fold_bn(
        np.asarray(inputs["w_c1"], np.float32),
        inputs["g_c1"], inputs["b_c1"], inputs["m_c1"], inputs["v_c1"],
    )
    w_c2, t_c2 = _fold_bn(
        np.asarray(inputs["w_c2"], np.float32),
        inputs["g_c2"], inputs["b_c2"], inputs["m_c2"], inputs["v_c2"],
    )
    w_p3, t_p3 = _fold_bn(
        np.asarray(inputs["w_p3"], np.float32),
        inputs["g_p3"], inputs["b_p3"], inputs["m_p3"], inputs["v_p3"],
    )

    # weight layouts (see build_nc): contraction channel on partitions
    def lay3x3(wf, cout_half):
        # wf [O, I, 3, 3] -> [128ci, n_i_sub, (oh?), 9, 128co]
        O, I = wf.shape[0], wf.shape[1]
        a = wf.reshape(O // P, P, I // P, P, 3, 3)  # [ohs, co, s, ci, ky, kx]
        a = a.transpose(3, 0, 2, 4, 5, 1)  # [ci, ohs, s, ky, kx, co]
        return np.ascontiguousarray(a)

    wp1a = lay3x3(w_p1, 1).reshape(P, 1, 2, 9, P)[:, 0]          # [128,2,9,128]
    wp2a = lay3x3(w_p2, 1).reshape(P, 1, 2, 9, P)[:, 0]
    wc1a = lay3x3(w_c1, 2).reshape(P, 2, 1, 9, P)[:, :, 0]       # [128,2oh,9,128]
    wp3a = lay3x3(w_p3, 2)                                        # [128,2oh,2s,9,128]
    wc2a = np.ascontiguousarray(
        w_c2[:, :, 0, 0].reshape(2, P, 2, P).transpose(3, 2, 0, 1)
    )  # [128ci, 2s, 2oh, 128co]

    bias = np.zeros((P, 6), np.float32)
    bias[:, 0] = t_p1
    bias[:, 1] = t_p2
    bc = t_c1 + t_c2
    bias[:, 2] = bc[:P]
    bias[:, 3] = bc[P:]
    bias[:, 4] = t_p3[:P]
    bias[:, 5] = t_p3[P:]

    # x slabs per core, W-padded + H halo, rounded to f32r
    xr = x.reshape(B, 2, P, H, W)
    slabs = np.zeros((B, 2, 2, P, HS, WP), np.float32)  # [b, half, s, p, h, w]
    for b in range(B):
        slabs[b, 0, :, :, 1:HS, 1 : WP - 1] = xr[b, :, :, 0:65, :]
        slabs[b, 1, :, :, 0 : HS - 1, 1 : WP - 1] = xr[b, :, :, 63:128, :]
    slabs = _r12(slabs)

    wmaps = {
        "wp1": _r12(wp1a),
        "wp2": _r12(wp2a),
        "wc1": _r12(wc1a),
        "wc2": _r12(wc2a),
        "wp3": _r12(wp3a),
    }
    in_maps = []
    for b in range(B):
        for half in range(2):
            bmv = np.zeros((P, 9), np.float32)
            bmv[:, 0:6] = bias
            if half == 0:  # top
                bmv[:, 6] = 1.0  # m_top
                bmv[:, 8] = 0.0
            else:  # bottom
                bmv[:, 7] = 1.0  # m_bot
                bmv[:, 8] = -1e30
            in_maps.append({"xs": slabs[b, half], "bm": bmv, **wmaps})

    global _last_in_maps
    _last_in_maps = in_maps

    nc = _get_nc()
    res = run_bass_kernel_spmd(nc, in_maps, list(range(8)))

    out = np.empty((B, CIN, H, W), np.float32)
    for b in range(B):
        out[b, :, 0:OH] = res.results[2 * b]["o"].reshape(CIN, OH, W)
        out[b, :, OH:H] = res.results[2 * b + 1]["o"].reshape(CIN, OH, W)
    return out


if __name__ == "__main__":
    import reference

    inp = {k: np.asarray(v) for k, v in reference.setup_inputs().items()}
    exp = np.asarray(reference.reference(**inp))
    got = kernel(**inp)
    err = np.abs(got - exp)
    rel = err.max() / max(np.abs(exp).max(), 1e-6)
    print("abs err max:", err.max(), "rel (vs absmax):", rel)



# revision 14
# speedup vs baseline: 1.1433x; 1.1433x over previous
"""Corner-pooling module kernel for 8 Trainium2 NeuronCores.

Reference computation (NCHW, fp32):
    p1 = relu(bn(conv3x3(x, w_p1)))          # 256 -> 128 ch
    p2 = relu(bn(conv3x3(x, w_p2)))          # 256 -> 128 ch
    cp1 = cummax(p1, axis=H, reverse=True)
    cp2 = cummax(p2, axis=W, reverse=True)
    r  = relu(bn(conv3x3(cp1+cp2, w_c1)) + bn(conv1x1(x, w_c2)))
    out = relu(bn(conv3x3(r, w_p3)))

Sharding: 8 cores = 4 samples x 2 H-halves.  BN folded into conv
weights host-side.

This version: 1D Winograd F(2,3) along W for every 3x3 conv, fp16
operands.  Each output-column PAIR costs 4 matmul columns per (ky,
input-slab) instead of 6, cutting PE work 1.5x; fp16 weights halve
LDWEIGHTS time (the co-bottleneck of the fp32r version).  Activations
live as even/odd column planes [P, rows, 66] (col 0/65 zero pads), so
the per-layer input transform (4 tensor ops/row-range, t0=O[c-1]-O[c],
t1=E+O, t2=O-E, t3=E[c]-E[c+1]) and the output combine (y0=m0+m1+m2,
y1=m1-m2-m3, via one ACT copy of m1 + 2 DVE + 2 DVE/GPSIMD tensor ops
+ 2 ACT relu's per chunk) are all unit-stride.  x's t-planes and E/O
planes are precomputed host-side.  Corner pooling in plane form:
H-cummax = two independent row chains (E,O); W-cummax = in-place
z=max(O,shift(E)), masked reverse flat scan, E=max(E,O).  The halo
exchange/colmax AllReduce machinery is the same as the fp32r version,
on 132-wide (E|O) rows.  Output is written fp16 and cast on host.
"""

import sys

sys.path.insert(0, "/opt/trn_rl_repo")

import numpy as np

import bass_rust
import concourse.bass as bass
import concourse.mybir as mybir
from concourse.bass_utils import run_bass_kernel_spmd
from concourse.tile import TileContext

F32 = mybir.dt.float32
F16 = mybir.dt.float16
EPS = 1e-5
AL = mybir.AluOpType
AF = mybir.ActivationFunctionType

B, CIN, H, W = 4, 256, 128, 128
P = 128
OH = 64             # own rows per core
C = 66              # plane cols: pad + 64 pairs + pad
HS = 66             # x/r plane rows: own 64 + 1 halo each side
SS = 68             # s plane rows: own 64 + 2 halo each side
NPC = 4
PCR = OH // NPC
NT = 512            # psum plane width / chunk size (flat pairs)
FLAT_P = OH * C     # 4224, p1/p2/p3 out flat size
FLAT_C = (OH + 2) * C  # 4356, c1 out flat size


def _chunks(total, nt):
    out = []
    q = 0
    while q < total:
        out.append((q, min(nt, total - q)))
        q += nt
    return out


def _row_chunks(row_list):
    # [(r0, nrows)] -> [(q, nt)] row-aligned
    return [(r0 * C, nr * C) for r0, nr in row_list]


def _fix_multiwaits(nc):
    """walrus in this container accepts at most ONE sem wait per
    instruction; split extras onto same-engine nops placed just before."""

    def steal_nop(eng):
        bi = nc.engines[eng].nop()
        ins = bi.ins
        cur = nc.cur_bb.bb
        lst = cur.instructions
        assert lst[-1] is ins or lst[-1].name == ins.name
        cur.instructions = lst[:-1]
        return ins

    for fn in nc.m.functions:
        for bb in fn.blocks:
            out = []
            changed = False
            for inst in bb.instructions:
                si = inst.sync_info
                waits = list(si.on_wait) if si is not None and si.on_wait else []
                if len(waits) > 1:
                    for wv in waits[:-1]:
                        nop = steal_nop(inst.engine)
                        nop.sync_info = bass_rust.SyncInfo(on_wait=[wv], on_update=[])
                        out.append(nop)
                    inst.sync_info = bass_rust.SyncInfo(
                        on_wait=[waits[-1]], on_update=list(si.on_update or [])
                    )
                    changed = True
                out.append(inst)
            if changed:
                bb.instructions = out
    return nc


def build_nc():
    nc = bass.Bass()

    xt = nc.dram_tensor("xt", [2, P, 4, HS, C], F16, kind="ExternalInput")
    xe = nc.dram_tensor("xe", [2, P, 2, HS, C], F16, kind="ExternalInput")
    gp1 = nc.dram_tensor("gp1", [P, 2, 3, 4, P], F16, kind="ExternalInput")
    gp2 = nc.dram_tensor("gp2", [P, 2, 3, 4, P], F16, kind="ExternalInput")
    gc1 = nc.dram_tensor("gc1", [P, 3, 4, 2, P], F16, kind="ExternalInput")
    gc2 = nc.dram_tensor("gc2", [P, 2, 2, P], F16, kind="ExternalInput")
    gp3 = nc.dram_tensor("gp3", [P, 2, 2, 3, 4, P], F16, kind="ExternalInput")
    bm = nc.dram_tensor("bm", [P, 12], F32, kind="ExternalInput")
    o = nc.dram_tensor("o", [2, P, OH, W], F16, kind="ExternalOutput")

    groups = [[0, 1], [2, 3], [4, 5], [6, 7]]

    with TileContext(nc) as tc:
        with (
            tc.tile_pool(name="const", bufs=1) as cpool,
            tc.tile_pool(name="wt", bufs=1) as wpool,
            tc.tile_pool(name="psum", bufs=2, space="PSUM") as psum,
            tc.tile_pool(name="epi", bufs=2) as epool,
            tc.tile_pool(name="scratch", bufs=1) as spool_s,
            tc.tile_pool(name="dram", bufs=1, space="DRAM") as dpool,
            tc.tile_pool(name="s", bufs=1) as spool,
            tc.tile_pool(name="x", bufs=1) as xpool,
        ):
            bmt = cpool.tile([P, 12], F32)
            nc.sync.dma_start(bmt[:], bm[:])
            m_top = bmt[:, 6:7]
            m_bot = bmt[:, 7:8]
            coloff = bmt[:, 8:9]

            # ---- weights: gp2 first (prewarm + early p2 chunks) ----
            gp1t = wpool.tile([P, 2, 3, 4, P], F16, tag="wg1")
            gp2t = wpool.tile([P, 2, 3, 4, P], F16, tag="wg2")
            nc.sync.dma_start(gp2t[:, 0], gp2[:, 0])
            nc.sync.dma_start(gp2t[:, 1], gp2[:, 1])
            gc1t = wpool.tile([P, 3, 4, 2, P], F16, tag="wgc1")
            gc2t = wpool.tile([P, 2, 2, P], F16, tag="wgc2")

            # ---- PE pre-warm on gp2 ----
            gp2fl = gp2t[:].rearrange("p a b c d -> p (a b c d)")
            ptw = psum.tile([P, 4, NT], F32, tag="ps")
            for _ in range(3):
                nc.tensor.matmul(
                    ptw[:, 0, :], gp2fl[:, 0:P], gp2fl[:, 0:NT],
                    start=True, stop=True,
                )

            # ---- x t-planes + E/O planes ----
            xtt = [xpool.tile([P, 4, HS, C], F16, name=f"xt{s}", tag=f"xt{s}")
                   for s in range(2)]
            xet = [xpool.tile([P, 2, HS, C], F16, name=f"xe{s}", tag=f"xe{s}")
                   for s in range(2)]
            starts = [60, 0, 6, 54, 48, 42, 36, 30, 24, 18, 12]
            for k, r0 in enumerate(starts):
                for s in range(2):
                    nc.sync.dma_start(
                        xtt[s][:, :, r0:r0 + 6, :], xt[s, :, :, r0:r0 + 6, :]
                    )
                if k == 1:
                    nc.sync.dma_start(gp1t[:], gp1[:])
                elif k == 2:
                    nc.sync.dma_start(gc2t[:], gc2[:])
                elif k == 3:
                    nc.sync.dma_start(gc1t[:], gc1[:])
                elif k == 5:
                    nc.sync.dma_start(xet[0][:], xe[0])
                elif k == 7:
                    nc.sync.dma_start(xet[1][:], xe[1])
            xtf = [[xtt[s][:, w4].rearrange("p h w -> p (h w)") for w4 in range(4)]
                   for s in range(2)]
            xef = [[xet[s][:, eo].rearrange("p h w -> p (h w)") for eo in range(2)]
                   for s in range(2)]

            # ---- s planes (p1 output rows 2..65; halo rows 0,1,66,67) ----
            sE = spool.tile([P, SS, C], F16, name="sE", tag="sE")
            sO = spool.tile([P, SS, C], F16, name="sO", tag="sO")
            sEf = sE[:].rearrange("p h w -> p (h w)")
            sOf = sO[:].rearrange("p h w -> p (h w)")

            def epilogue(pt, nt, bias, dstE, dstO, alt):
                """y0 = relu(m0+m1+m2+b) -> dstE, y1 = relu(m1-m2-m3+b) -> dstO.
                alt picks the engine for the y1 chain (balance DVE/GPSIMD)."""
                ta = epool.tile([P, NT], F32, tag="ta")
                tb = epool.tile([P, NT], F32, tag="tb")
                nc.scalar.activation(ta[:, :nt], pt[:, 1, :nt], AF.Copy)
                nc.vector.tensor_tensor(tb[:, :nt], ta[:, :nt], pt[:, 0, :nt], AL.add)
                nc.vector.tensor_tensor(tb[:, :nt], tb[:, :nt], pt[:, 2, :nt], AL.add)
                nc.scalar.activation(dstE, tb[:, :nt], AF.Relu, bias=bias)
                # GPSIMD cannot touch PSUM, so the y1 chain stays on DVE
                nc.vector.tensor_tensor(ta[:, :nt], ta[:, :nt], pt[:, 2, :nt],
                                        AL.subtract)
                nc.vector.tensor_tensor(ta[:, :nt], ta[:, :nt], pt[:, 3, :nt],
                                        AL.subtract)
                nc.scalar.activation(dstO, ta[:, :nt], AF.Relu, bias=bias)

            def conv_p(gt, bias, outEf, outOf, out_base, chunk_list,
                       post_chunk=None):
                """p1/p2-style conv: K=256 (2 slabs), 128 out ch."""
                for i, (q, nt) in enumerate(chunk_list):
                    pt = psum.tile([P, 4, NT], F32, tag="ps")
                    for w4 in range(4):
                        terms = [(gt[:, s, ky, w4, :], xtf[s][w4], ky * C)
                                 for s in range(2) for ky in range(3)]
                        for j, (lhsT, rf, off) in enumerate(terms):
                            nc.tensor.matmul(
                                pt[:, w4, :nt], lhsT, rf[:, q + off:q + off + nt],
                                start=(j == 0), stop=(j == len(terms) - 1),
                            )
                    epilogue(pt, nt, bias,
                             outEf[:, out_base + q:out_base + q + nt],
                             outOf[:, out_base + q:out_base + q + nt],
                             alt=(i % 2 == 0))
                    if post_chunk is not None:
                        post_chunk(q, nt)

            # ---- conv p2 exchange chunks (rows 62-63 and 0-6) first ----
            with tc.tile_pool(name="p2", bufs=1) as p2pool:
                # 66 rows (only 0..63 used) so the slot fits r-half0 later
                p2E = p2pool.tile([P, HS, C], F16, name="p2E", tag="p2E")
                p2O = p2pool.tile([P, HS, C], F16, name="p2O", tag="p2O")
                p2Ef = p2E[:].rearrange("p h w -> p (h w)")
                p2Of = p2O[:].rearrange("p h w -> p (h w)")
                mk = p2pool.tile([P, PCR, C], F16)
                nc.vector.memset(mk[:], 1.0)
                nc.vector.memset(mk[:, :, 0:1], 0.0)
                nc.vector.memset(mk[:, :, C - 1:C], 0.0)
                mkf = mk[:].rearrange("p h w -> p (h w)")

                p2_rows = ([(62, 2), (0, 7)]
                           + [(r, min(7, 61 - r + 1)) for r in range(7, 62, 7)])
                p2_chunks = _row_chunks(p2_rows)
                conv_p(gp2t, bmt[:, 1:2], p2Ef, p2Of, 0, p2_chunks[:2])

                def wscan(rows_ap_E, rows_ap_O, mask_f):
                    # in-place W reverse cummax on an E/O row range
                    nc.vector.tensor_tensor(
                        rows_ap_O[:, :, 1:65], rows_ap_O[:, :, 1:65],
                        rows_ap_E[:, :, 2:66], AL.max,
                    )
                    flatO = rows_ap_O.rearrange("p h w -> p (h w)")
                    nc.vector.tensor_tensor_scan(
                        flatO[:, ::-1], flatO[:, ::-1], mask_f[:, ::-1],
                        0.0, AL.max, AL.mult,
                    )
                    nc.vector.tensor_tensor(
                        rows_ap_E[:], rows_ap_E[:], rows_ap_O[:], AL.max,
                    )

                for r in (62, 63, 0, 1):
                    nc.vector.memset(p2E[:, r:r + 1, C - 1:C], 0.0)
                    wscan(p2E[:, r:r + 1, :], p2O[:, r:r + 1, :],
                          mkf[:, 0:C])

                # ---- conv p1 reverse order; H-chain interleaves under it ----
                # chain step h (s rows h,h+1 = out rows h-2,h-1) is emitted as
                # soon as the reverse chunk sweep covers out row h-2, so the
                # DVE runs it ~1 chunk behind the PE instead of after conv p1.
                p1_chunks = _chunks(FLAT_P, NT)
                chain_h = [OH]

                def p1_post(q, nt):
                    lo = max((q + C - 1) // C + 2, 2)
                    for h in range(chain_h[0], lo - 1, -1):
                        nc.vector.tensor_tensor(
                            sE[:, h, :], sE[:, h, :], sE[:, h + 1, :], AL.max)
                        nc.vector.tensor_tensor(
                            sO[:, h, :], sO[:, h, :], sO[:, h + 1, :], AL.max)
                    chain_h[0] = min(chain_h[0], lo - 1)

                conv_p(gp1t, bmt[:, 0:1], sEf, sOf, 2 * C,
                       list(reversed(p1_chunks)), post_chunk=p1_post)

                # ---- pairwise exchange (E|O concat, 132-wide rows) ----
                ct = spool_s.tile([P, 8, 2 * C], F16, tag="exch")
                srcs = [
                    (sE[:, 2, :], sO[:, 2, :], m_bot),
                    (sE[:, 3, :], sO[:, 3, :], m_bot),
                    (sE[:, 2 + OH - 2, :], sO[:, 2 + OH - 2, :], m_top),
                    (sE[:, 2 + OH - 1, :], sO[:, 2 + OH - 1, :], m_top),
                    (p2E[:, 0, :], p2O[:, 0, :], m_bot),
                    (p2E[:, 1, :], p2O[:, 1, :], m_bot),
                    (p2E[:, OH - 2, :], p2O[:, OH - 2, :], m_top),
                    (p2E[:, OH - 1, :], p2O[:, OH - 1, :], m_top),
                ]
                for k, (se_, so_, m) in enumerate(srcs):
                    nc.vector.tensor_scalar_mul(ct[:, k, 0:C], se_, m)
                    nc.vector.tensor_scalar_mul(ct[:, k, C:2 * C], so_, m)
                cc_in = dpool.tile([P, 8, 2 * C], F16)
                cc_out = dpool.tile([P, 8, 2 * C], F16)
                nc.sync.dma_start(cc_in[:], ct[:])
                nc.gpsimd.collective_compute(
                    "AllReduce", AL.add, replica_groups=groups,
                    ins=[cc_in[:]], outs=[cc_out[:]],
                )
                rx = spool_s.tile([P, 8, 2 * C], F16, tag="exch2")
                nc.sync.dma_start(rx[:], cc_out[:])

                # u = R[0] + coloff (top: partner colmax; bottom: -inf)
                u = spool_s.tile([P, 2 * C], F16, tag="u")
                nc.vector.tensor_scalar_add(u[:], rx[:, 0, :], coloff)

                # ---- conv p2 remaining chunks (overlap the collective) ----
                conv_p(gp2t, bmt[:, 1:2], p2Ef, p2Of, 0, p2_chunks[2:])

                # gp3 into the slots gp1/gp2 free after their last chunks
                # (emitted only now that every gp1/gp2 reader exists)
                gp3t = [wpool.tile([P, 2, 3, 4, P], F16, name=f"gp3{t}", tag=t)
                        for t in ("wg1", "wg2")]
                nc.sync.dma_start(gp3t[0][:], gp3[:, 0])
                nc.sync.dma_start(gp3t[1][:], gp3[:, 1])

                # ---- s-plane t-transform target ----
                with tc.tile_pool(name="st", bufs=1) as stpool:
                    st = stpool.tile([P, 4, SS, C], F16)
                    nc.vector.memset(st[:, :, :, 0:1], 0.0)
                    nc.vector.memset(st[:, :, :, C - 1:C], 0.0)
                    stf = [st[:, w4].rearrange("p h w -> p (h w)")
                           for w4 in range(4)]
                    cm = spool_s.tile([P, 2 * C], F16, tag="cm")
                    h0 = spool_s.tile([P, C], F16, tag="h0")
                    h1 = spool_s.tile([P, C], F16, tag="h1")

                    def st_xform(r0, nr):
                        # st rows r0..r0+nr from s rows (same tile rows)
                        args = [
                            (st[:, 0, r0:r0 + nr, 1:65],
                             sO[:, r0:r0 + nr, 0:64], sO[:, r0:r0 + nr, 1:65],
                             AL.subtract),
                            (st[:, 1, r0:r0 + nr, 1:65],
                             sE[:, r0:r0 + nr, 1:65], sO[:, r0:r0 + nr, 1:65],
                             AL.add),
                            (st[:, 2, r0:r0 + nr, 1:65],
                             sO[:, r0:r0 + nr, 1:65], sE[:, r0:r0 + nr, 1:65],
                             AL.subtract),
                            (st[:, 3, r0:r0 + nr, 1:65],
                             sE[:, r0:r0 + nr, 1:65], sE[:, r0:r0 + nr, 2:66],
                             AL.subtract),
                        ]
                        for i, (d, a, b_, op) in enumerate(args):
                            eng = nc.vector if i % 2 == 0 else nc.gpsimd
                            eng.tensor_tensor(d, a, b_, op)

                    for pc in range(NPC):
                        r0 = pc * PCR
                        sr0 = 2 + r0
                        nc.vector.memset(p2E[:, r0:r0 + PCR, C - 1:C], 0.0)
                        wscan(p2E[:, r0:r0 + PCR, :], p2O[:, r0:r0 + PCR, :],
                              mkf)
                        for sp_, uc in ((sE, 0), (sO, C)):
                            nc.vector.tensor_tensor(
                                sp_[:, sr0:sr0 + PCR, :],
                                sp_[:, sr0:sr0 + PCR, :],
                                u[:, None, uc:uc + C].to_broadcast((P, PCR, C)),
                                AL.max,
                            )
                        if pc == 0:
                            nc.vector.tensor_copy(cm[:, 0:C], sE[:, 2, :])
                            nc.vector.tensor_copy(cm[:, C:2 * C], sO[:, 2, :])
                        nc.gpsimd.tensor_tensor(
                            sE[:, sr0:sr0 + PCR, :], sE[:, sr0:sr0 + PCR, :],
                            p2E[:, r0:r0 + PCR, :], AL.add)
                        nc.gpsimd.tensor_tensor(
                            sO[:, sr0:sr0 + PCR, :], sO[:, sr0:sr0 + PCR, :],
                            p2O[:, r0:r0 + PCR, :], AL.add)
                        for sp_ in (sE, sO):
                            nc.vector.memset(sp_[:, sr0:sr0 + PCR, 0:1], 0.0)
                            nc.vector.memset(sp_[:, sr0:sr0 + PCR, C - 1:C], 0.0)
                        if pc == 0:
                            # halo rows: above (bottom cores) s rows 0,1
                            for j in range(2):
                                for sp_, cc0 in ((sE, 0), (sO, C)):
                                    nc.vector.tensor_tensor(
                                        h0[:], rx[:, 2 + j, cc0:cc0 + C],
                                        cm[:, cc0:cc0 + C], AL.max)
                                    nc.vector.tensor_tensor(
                                        h0[:], h0[:], rx[:, 6 + j, cc0:cc0 + C],
                                        AL.add)
                                    nc.vector.tensor_scalar_mul(
                                        sp_[:, j, :], h0[:], m_bot)
                            # below (top cores) s rows 66,67
                            for j in range(2):
                                for sp_, cc0 in ((sE, 0), (sO, C)):
                                    nc.vector.tensor_tensor(
                                        h1[:], rx[:, 0 + j, cc0:cc0 + C],
                                        rx[:, 4 + j, cc0:cc0 + C], AL.add)
                                    nc.vector.tensor_scalar_mul(
                                        sp_[:, SS - 2 + j, :], h1[:], m_top)
                            for sp_ in (sE, sO):
                                for rr in (0, SS - 2):
                                    nc.vector.memset(
                                        sp_[:, rr:rr + 2, 0:1], 0.0)
                                    nc.vector.memset(
                                        sp_[:, rr:rr + 2, C - 1:C], 0.0)
                            st_xform(0, 2)
                            st_xform(SS - 2, 2)
                        st_xform(sr0, PCR)

                    # ---- conv c1 (+ folded c2) -> r planes ----
                    c1_chunks = _chunks(FLAT_C, NT)
                    rpl = []
                    for half in range(2):
                        if half == 0:
                            rE = p2pool.tile([P, HS, C], F16, tag="p2E")
                            rO = p2pool.tile([P, HS, C], F16, tag="p2O")
                        else:
                            rE = spool.tile([P, HS, C], F16, tag="sE")
                            rO = spool.tile([P, HS, C], F16, tag="sO")
                        rEf = rE[:].rearrange("p h w -> p (h w)")
                        rOf = rO[:].rearrange("p h w -> p (h w)")
                        rpl.append((rE, rO, rEf, rOf))
                        for i, (q, nt) in enumerate(c1_chunks):
                            pt = psum.tile([P, 4, NT], F32, tag="ps")
                            for w4 in range(4):
                                terms = [(gc1t[:, ky, w4, half, :], stf[w4],
                                          ky * C) for ky in range(3)]
                                if w4 == 0:
                                    terms += [(gc2t[:, s, half, :], xef[s][0], 0)
                                              for s in range(2)]
                                elif w4 == 3:
                                    terms += [(gc2t[:, s, half, :], xef[s][1], 0)
                                              for s in range(2)]
                                for j, (lhsT, rf, off) in enumerate(terms):
                                    nc.tensor.matmul(
                                        pt[:, w4, :nt], lhsT,
                                        rf[:, q + off:q + off + nt],
                                        start=(j == 0),
                                        stop=(j == len(terms) - 1),
                                    )
                            epilogue(pt, nt, bmt[:, 2 + half:3 + half],
                                     rEf[:, q:q + nt], rOf[:, q:q + nt],
                                     alt=(i % 2 == 0))
                        # mask invalid halo rows, zero pads
                        for rp_ in (rE, rO):
                            nc.vector.tensor_scalar_mul(
                                rp_[:, 0, :], rp_[:, 0, :], m_bot)
                            nc.vector.tensor_scalar_mul(
                                rp_[:, HS - 1, :], rp_[:, HS - 1, :], m_top)
                            nc.vector.memset(rp_[:, :, 0:1], 0.0)
                            nc.vector.memset(rp_[:, :, C - 1:C], 0.0)

                    # ---- r t-planes (into the xt slots) ----
                    rtt = []
                    for half in range(2):
                        rt_ = xpool.tile([P, 4, HS, C], F16, tag=f"xt{half}")
                        nc.vector.memset(rt_[:, :, :, 0:1], 0.0)
                        nc.vector.memset(rt_[:, :, :, C - 1:C], 0.0)
                        rtt.append(rt_)
                    for half in range(2):
                        rE, rO, _, _ = rpl[half]
                        rt_ = rtt[half]
                        for r0, nr in ((0, 17), (17, 17), (34, 16), (50, 16)):
                            args = [
                                (rt_[:, 0, r0:r0 + nr, 1:65],
                                 rO[:, r0:r0 + nr, 0:64],
                                 rO[:, r0:r0 + nr, 1:65], AL.subtract),
                                (rt_[:, 1, r0:r0 + nr, 1:65],
                                 rE[:, r0:r0 + nr, 1:65],
                                 rO[:, r0:r0 + nr, 1:65], AL.add),
                                (rt_[:, 2, r0:r0 + nr, 1:65],
                                 rO[:, r0:r0 + nr, 1:65],
                                 rE[:, r0:r0 + nr, 1:65], AL.subtract),
                                (rt_[:, 3, r0:r0 + nr, 1:65],
                                 rE[:, r0:r0 + nr, 1:65],
                                 rE[:, r0:r0 + nr, 2:66], AL.subtract),
                            ]
                            for i, (d, a, b_, op) in enumerate(args):
                                eng = nc.vector if i % 2 == 0 else nc.gpsimd
                                eng.tensor_tensor(d, a, b_, op)
                    rtf = [[rtt[s][:, w4].rearrange("p h w -> p (h w)")
                            for w4 in range(4)] for s in range(2)]

                    # ---- conv p3 -> interleaved fp16 staging -> out ----
                    p3_rows = [(r, 6) for r in range(0, 60, 6)] + [(60, 4)]
                    for half in range(2):
                        stg = xpool.tile([P, OH, W], F16, tag=f"xe{half}")
                        for i, (r0, nr) in enumerate(p3_rows):
                            q, nt = r0 * C, nr * C
                            pt = psum.tile([P, 4, NT], F32, tag="ps")
                            for w4 in range(4):
                                terms = [(gp3t[s][:, half, ky, w4, :],
                                          rtf[s][w4], ky * C)
                                         for s in range(2) for ky in range(3)]
                                for j, (lhsT, rf, off) in enumerate(terms):
                                    nc.tensor.matmul(
                                        pt[:, w4, :nt], lhsT,
                                        rf[:, q + off:q + off + nt],
                                        start=(j == 0),
                                        stop=(j == len(terms) - 1),
                                    )
                            ta = epool.tile([P, 6, C], F32, tag="ta")
                            tb = epool.tile([P, 6, C], F32, tag="tb")
                            taf = ta[:].rearrange("p h w -> p (h w)")
                            tbf = tb[:].rearrange("p h w -> p (h w)")
                            bias = bmt[:, 4 + half:5 + half]
                            nc.scalar.activation(taf[:, :nt], pt[:, 1, :nt],
                                                 AF.Copy)
                            nc.vector.tensor_tensor(
                                tbf[:, :nt], taf[:, :nt], pt[:, 0, :nt], AL.add)
                            nc.vector.tensor_tensor(
                                tbf[:, :nt], tbf[:, :nt], pt[:, 2, :nt], AL.add)
                            nc.scalar.activation(
                                stg[:, r0:r0 + nr, 0:W:2],
                                tb[:, :nr, 1:65], AF.Relu, bias=bias)
                            nc.vector.tensor_tensor(
                                taf[:, :nt], taf[:, :nt], pt[:, 2, :nt],
                                AL.subtract)
                            nc.vector.tensor_tensor(
                                taf[:, :nt], taf[:, :nt], pt[:, 3, :nt],
                                AL.subtract)
                            nc.scalar.activation(
                                stg[:, r0:r0 + nr, 1:W:2],
                                ta[:, :nr, 1:65], AF.Relu, bias=bias)
                        for r0 in range(0, OH, 8):
                            nc.sync.dma_start(
                                o[half, :, r0:r0 + 8, :], stg[:, r0:r0 + 8, :])

    _fix_multiwaits(nc)
    return nc


_NC = None


def _get_nc():
    global _NC
    if _NC is None:
        _NC = build_nc()
    return _NC


def _fold_bn(w, g, b, m, v):
    s = (np.asarray(g) / np.sqrt(np.asarray(v) + EPS)).astype(np.float32)
    t = (np.asarray(b) - np.asarray(m) * s).astype(np.float32)
    return np.asarray(w, np.float32) * s[:, None, None, None], t


def _wino_w(w):
    # w [O, I, 3, 3] -> G [4, 3ky, I, O]
    g0, g1, g2 = w[..., 0], w[..., 1], w[..., 2]
    G = np.stack([g0, (g0 + g1 + g2) * 0.5, (g0 - g1 + g2) * 0.5, g2])
    return G.transpose(0, 3, 2, 1).astype(np.float16)


def kernel(**inputs):
    x = np.asarray(inputs["x"], np.float32)

    w_p1, t_p1 = _fold_bn(inputs["w_p1"], inputs["g_p1"], inputs["b_p1"],
                          inputs["m_p1"], inputs["v_p1"])
    w_p2, t_p2 = _fold_bn(inputs["w_p2"], inputs["g_p2"], inputs["b_p2"],
                          inputs["m_p2"], inputs["v_p2"])
    w_c1, t_c1 = _fold_bn(inputs["w_c1"], inputs["g_c1"], inputs["b_c1"],
                          inputs["m_c1"], inputs["v_c1"])
    w_c2, t_c2 = _fold_bn(inputs["w_c2"], inputs["g_c2"], inputs["b_c2"],
                          inputs["m_c2"], inputs["v_c2"])
    w_p3, t_p3 = _fold_bn(inputs["w_p3"], inputs["g_p3"], inputs["b_p3"],
                          inputs["m_p3"], inputs["v_p3"])

    Gp1 = _wino_w(w_p1)  # [4,3,256,128]
    Gp2 = _wino_w(w_p2)
    Gc1 = _wino_w(w_c1)  # [4,3,128,256]
    Gp3 = _wino_w(w_p3)  # [4,3,256,256]

    gp1a = np.ascontiguousarray(
        Gp1.reshape(4, 3, 2, P, P).transpose(3, 2, 1, 0, 4))
    gp2a = np.ascontiguousarray(
        Gp2.reshape(4, 3, 2, P, P).transpose(3, 2, 1, 0, 4))
    gc1a = np.ascontiguousarray(
        Gc1.reshape(4, 3, P, 2, P).transpose(2, 1, 0, 3, 4))
    gp3a = np.ascontiguousarray(
        Gp3.reshape(4, 3, 2, P, 2, P).transpose(3, 2, 4, 1, 0, 5))
    gc2a = np.ascontiguousarray(
        w_c2[:, :, 0, 0].reshape(2, P, 2, P).transpose(3, 2, 0, 1)
    ).astype(np.float16)

    bias = np.zeros((P, 6), np.float32)
    bias[:, 0] = t_p1
    bias[:, 1] = t_p2
    bc = t_c1 + t_c2
    bias[:, 2] = bc[:P]
    bias[:, 3] = bc[P:]
    bias[:, 4] = t_p3[:P]
    bias[:, 5] = t_p3[P:]

    # x slabs per core-half with H halo, as fp16 E/O planes + t-planes
    x16 = x.astype(np.float16).astype(np.float32)
    xr = x16.reshape(B, 2, P, H, W)
    pad = np.zeros((B, 2, 2, P, HS, W), np.float32)  # [b, half, slab, p, h, w]
    pad[:, 0, :, :, 1:HS, :] = xr[:, :, :, 0:65, :]
    pad[:, 1, :, :, 0:HS - 1, :] = xr[:, :, :, 63:128, :]
    xE = np.zeros((B, 2, 2, P, HS, C), np.float32)
    xO = np.zeros_like(xE)
    xE[..., 1:65] = pad[..., 0::2]
    xO[..., 1:65] = pad[..., 1::2]
    t4 = np.zeros((B, 2, 2, P, 4, HS, C), np.float32)
    t4[..., 0, :, 1:65] = xO[..., 0:64] - xO[..., 1:65]
    t4[..., 1, :, 1:65] = xE[..., 1:65] + xO[..., 1:65]
    t4[..., 2, :, 1:65] = xO[..., 1:65] - xE[..., 1:65]
    t4[..., 3, :, 1:65] = xE[..., 1:65] - xE[..., 2:66]
    t4 = t4.astype(np.float16)
    xeo = np.stack([xE, -xO], axis=4).astype(np.float16)  # [b,half,slab,p,2,h,c]

    wmaps = {"gp1": gp1a, "gp2": gp2a, "gc1": gc1a, "gc2": gc2a, "gp3": gp3a}
    in_maps = []
    for b in range(B):
        for half in range(2):
            bmv = np.zeros((P, 12), np.float32)
            bmv[:, 0:6] = bias
            if half == 0:  # top
                bmv[:, 6] = 1.0
                bmv[:, 8] = 0.0
            else:  # bottom
                bmv[:, 7] = 1.0
                bmv[:, 8] = -1e30
            in_maps.append({
                "xt": t4[b, half], "xe": xeo[b, half], "bm": bmv, **wmaps,
            })

    global _last_in_maps
    _last_in_maps = in_maps

    nc = _get_nc()
    res = run_bass_kernel_spmd(nc, in_maps, list(range(8)))

    out = np.empty((B, CIN, H, W), np.float32)
    for b in range(B):
        out[b, :, 0:OH] = res.results[2 * b]["o"].reshape(CIN, OH, W)
        out[b, :, OH:H] = res.results[2 * b + 1]["o"].reshape(CIN, OH, W)
    return out


if __name__ == "__main__":
    import reference

    inp = {k: np.asarray(v) for k, v in reference.setup_inputs().items()}
    exp = np.asarray(reference.reference(**inp))
    got = kernel(**inp)
    err = np.abs(got - exp)
    rel = err.max() / max(np.abs(exp).max(), 1e-6)
    print("abs err max:", err.max(), "rel (vs absmax):", rel)


# revision 20
# speedup vs baseline: 1.1464x; 1.0027x over previous
"""Corner-pooling module kernel for 8 Trainium2 NeuronCores.

Reference computation (NCHW, fp32):
    p1 = relu(bn(conv3x3(x, w_p1)))          # 256 -> 128 ch
    p2 = relu(bn(conv3x3(x, w_p2)))          # 256 -> 128 ch
    cp1 = cummax(p1, axis=H, reverse=True)
    cp2 = cummax(p2, axis=W, reverse=True)
    r  = relu(bn(conv3x3(cp1+cp2, w_c1)) + bn(conv1x1(x, w_c2)))
    out = relu(bn(conv3x3(r, w_p3)))

Sharding: 8 cores = 4 samples x 2 H-halves.  BN folded into conv
weights host-side.

This version: 1D Winograd F(2,3) along W for every 3x3 conv, fp16
operands.  Each output-column PAIR costs 4 matmul columns per (ky,
input-slab) instead of 6, cutting PE work 1.5x; fp16 weights halve
LDWEIGHTS time (the co-bottleneck of the fp32r version).  Activations
live as even/odd column planes [P, rows, 66] (col 0/65 zero pads), so
the per-layer input transform (4 tensor ops/row-range, t0=O[c-1]-O[c],
t1=E+O, t2=O-E, t3=E[c]-E[c+1]) and the output combine (y0=m0+m1+m2,
y1=m1-m2-m3, via one ACT copy of m1 + 2 DVE + 2 DVE/GPSIMD tensor ops
+ 2 ACT relu's per chunk) are all unit-stride.  x's t-planes and E/O
planes are precomputed host-side.  Corner pooling in plane form:
H-cummax = two independent row chains (E,O); W-cummax = in-place
z=max(O,shift(E)), masked reverse flat scan, E=max(E,O).  The halo
exchange/colmax AllReduce machinery is the same as the fp32r version,
on 132-wide (E|O) rows.  Output is written fp16 and cast on host.
"""

import sys

sys.path.insert(0, "/opt/trn_rl_repo")

import numpy as np

import bass_rust
import concourse.bass as bass
import concourse.mybir as mybir
from concourse.bass_utils import run_bass_kernel_spmd
from concourse.tile import TileContext

F32 = mybir.dt.float32
F16 = mybir.dt.float16
EPS = 1e-5
AL = mybir.AluOpType
AF = mybir.ActivationFunctionType

B, CIN, H, W = 4, 256, 128, 128
P = 128
OH = 64             # own rows per core
C = 66              # plane cols: pad + 64 pairs + pad
HS = 66             # x/r plane rows: own 64 + 1 halo each side
SS = 68             # s plane rows: own 64 + 2 halo each side
NPC = 4
PCR = OH // NPC
NT = 512            # psum plane width / chunk size (flat pairs)
FLAT_P = OH * C     # 4224, p1/p2/p3 out flat size
FLAT_C = (OH + 2) * C  # 4356, c1 out flat size


def _chunks(total, nt):
    out = []
    q = 0
    while q < total:
        out.append((q, min(nt, total - q)))
        q += nt
    return out


def _row_chunks(row_list):
    # [(r0, nrows)] -> [(q, nt)] row-aligned
    return [(r0 * C, nr * C) for r0, nr in row_list]


def _fix_multiwaits(nc):
    """walrus in this container accepts at most ONE sem wait per
    instruction; split extras onto same-engine nops placed just before."""

    def steal_nop(eng):
        bi = nc.engines[eng].nop()
        ins = bi.ins
        cur = nc.cur_bb.bb
        lst = cur.instructions
        assert lst[-1] is ins or lst[-1].name == ins.name
        cur.instructions = lst[:-1]
        return ins

    for fn in nc.m.functions:
        for bb in fn.blocks:
            out = []
            changed = False
            for inst in bb.instructions:
                si = inst.sync_info
                waits = list(si.on_wait) if si is not None and si.on_wait else []
                if len(waits) > 1:
                    for wv in waits[:-1]:
                        nop = steal_nop(inst.engine)
                        nop.sync_info = bass_rust.SyncInfo(on_wait=[wv], on_update=[])
                        out.append(nop)
                    inst.sync_info = bass_rust.SyncInfo(
                        on_wait=[waits[-1]], on_update=list(si.on_update or [])
                    )
                    changed = True
                out.append(inst)
            if changed:
                bb.instructions = out
    return nc


def build_nc():
    nc = bass.Bass()

    xt = nc.dram_tensor("xt", [2, P, 4, HS, C], F16, kind="ExternalInput")
    xe = nc.dram_tensor("xe", [2, P, 2, HS, C], F16, kind="ExternalInput")
    gp1 = nc.dram_tensor("gp1", [P, 2, 3, 4, P], F16, kind="ExternalInput")
    gp2 = nc.dram_tensor("gp2", [P, 2, 3, 4, P], F16, kind="ExternalInput")
    gc1 = nc.dram_tensor("gc1", [P, 3, 4, 2, P], F16, kind="ExternalInput")
    gc2 = nc.dram_tensor("gc2", [P, 2, 2, P], F16, kind="ExternalInput")
    gp3 = nc.dram_tensor("gp3", [P, 2, 2, 3, 4, P], F16, kind="ExternalInput")
    bm = nc.dram_tensor("bm", [P, 12], F32, kind="ExternalInput")
    o = nc.dram_tensor("o", [2, P, OH, W], F16, kind="ExternalOutput")

    groups = [[0, 1], [2, 3], [4, 5], [6, 7]]

    with TileContext(nc) as tc:
        with (
            tc.tile_pool(name="const", bufs=1) as cpool,
            tc.tile_pool(name="wt", bufs=1) as wpool,
            tc.tile_pool(name="psum", bufs=2, space="PSUM") as psum,
            tc.tile_pool(name="epi", bufs=2) as epool,
            tc.tile_pool(name="scratch", bufs=1) as spool_s,
            tc.tile_pool(name="dram", bufs=1, space="DRAM") as dpool,
            tc.tile_pool(name="s", bufs=1) as spool,
            tc.tile_pool(name="x", bufs=1) as xpool,
        ):
            bmt = cpool.tile([P, 12], F32)
            nc.sync.dma_start(bmt[:], bm[:])
            m_top = bmt[:, 6:7]
            m_bot = bmt[:, 7:8]
            coloff = bmt[:, 8:9]

            # ---- weights: gp2 first (prewarm + early p2 chunks) ----
            gp1t = wpool.tile([P, 2, 3, 4, P], F16, tag="wg1")
            gp2t = wpool.tile([P, 2, 3, 4, P], F16, tag="wg2")
            nc.sync.dma_start(gp2t[:, 0], gp2[:, 0])
            nc.sync.dma_start(gp2t[:, 1], gp2[:, 1])
            gc1t = wpool.tile([P, 3, 4, 2, P], F16, tag="wgc1")
            gc2t = wpool.tile([P, 2, 2, P], F16, tag="wgc2")

            # ---- PE pre-warm on gp2 ----
            gp2fl = gp2t[:].rearrange("p a b c d -> p (a b c d)")
            ptw = psum.tile([P, 4, NT], F32, tag="ps")
            for _ in range(3):
                nc.tensor.matmul(
                    ptw[:, 0, :], gp2fl[:, 0:P], gp2fl[:, 0:NT],
                    start=True, stop=True,
                )

            # ---- x t-planes + E/O planes ----
            xtt = [xpool.tile([P, 4, HS, C], F16, name=f"xt{s}", tag=f"xt{s}")
                   for s in range(2)]
            xet = [xpool.tile([P, 2, HS, C], F16, name=f"xe{s}", tag=f"xe{s}")
                   for s in range(2)]
            starts = [60, 0, 6, 54, 48, 42, 36, 30, 24, 18, 12]
            for k, r0 in enumerate(starts):
                for s in range(2):
                    nc.sync.dma_start(
                        xtt[s][:, :, r0:r0 + 6, :], xt[s, :, :, r0:r0 + 6, :]
                    )
                if k == 1:
                    nc.sync.dma_start(gp1t[:], gp1[:])
                elif k == 2:
                    nc.sync.dma_start(gc2t[:], gc2[:])
                elif k == 3:
                    nc.sync.dma_start(gc1t[:], gc1[:])
                elif k == 5:
                    nc.sync.dma_start(xet[0][:], xe[0])
                elif k == 7:
                    nc.sync.dma_start(xet[1][:], xe[1])
            xtf = [[xtt[s][:, w4].rearrange("p h w -> p (h w)") for w4 in range(4)]
                   for s in range(2)]
            xef = [[xet[s][:, eo].rearrange("p h w -> p (h w)") for eo in range(2)]
                   for s in range(2)]

            # ---- s planes (p1 output rows 2..65; halo rows 0,1,66,67) ----
            sE = spool.tile([P, SS, C], F16, name="sE", tag="sE")
            sO = spool.tile([P, SS, C], F16, name="sO", tag="sO")
            sEf = sE[:].rearrange("p h w -> p (h w)")
            sOf = sO[:].rearrange("p h w -> p (h w)")

            def epilogue(pt, nt, bias, dstE, dstO, alt):
                """y0 = relu(m0+m1+m2+b) -> dstE, y1 = relu(m1-m2-m3+b) -> dstO.
                alt picks the engine for the y1 chain (balance DVE/GPSIMD)."""
                ta = epool.tile([P, NT], F32, tag="ta")
                tb = epool.tile([P, NT], F32, tag="tb")
                nc.scalar.activation(ta[:, :nt], pt[:, 1, :nt], AF.Copy)
                nc.vector.tensor_tensor(tb[:, :nt], ta[:, :nt], pt[:, 0, :nt], AL.add)
                nc.vector.tensor_tensor(tb[:, :nt], tb[:, :nt], pt[:, 2, :nt], AL.add)
                nc.scalar.activation(dstE, tb[:, :nt], AF.Relu, bias=bias)
                # GPSIMD cannot touch PSUM, so the y1 chain stays on DVE
                nc.vector.tensor_tensor(ta[:, :nt], ta[:, :nt], pt[:, 2, :nt],
                                        AL.subtract)
                nc.vector.tensor_tensor(ta[:, :nt], ta[:, :nt], pt[:, 3, :nt],
                                        AL.subtract)
                nc.scalar.activation(dstO, ta[:, :nt], AF.Relu, bias=bias)

            def conv_p(gt, bias, outEf, outOf, out_base, chunk_list,
                       post_chunk=None):
                """p1/p2-style conv: K=256 (2 slabs), 128 out ch."""
                for i, (q, nt) in enumerate(chunk_list):
                    pt = psum.tile([P, 4, NT], F32, tag="ps")
                    for w4 in range(4):
                        terms = [(gt[:, s, ky, w4, :], xtf[s][w4], ky * C)
                                 for s in range(2) for ky in range(3)]
                        for j, (lhsT, rf, off) in enumerate(terms):
                            nc.tensor.matmul(
                                pt[:, w4, :nt], lhsT, rf[:, q + off:q + off + nt],
                                start=(j == 0), stop=(j == len(terms) - 1),
                            )
                    epilogue(pt, nt, bias,
                             outEf[:, out_base + q:out_base + q + nt],
                             outOf[:, out_base + q:out_base + q + nt],
                             alt=(i % 2 == 0))
                    if post_chunk is not None:
                        post_chunk(q, nt)

            # ---- conv p2 exchange chunks (rows 62-63 and 0-6) first ----
            with tc.tile_pool(name="p2", bufs=1) as p2pool:
                # 66 rows (only 0..63 used) so the slot fits r-half0 later
                p2E = p2pool.tile([P, HS, C], F16, name="p2E", tag="p2E")
                p2O = p2pool.tile([P, HS, C], F16, name="p2O", tag="p2O")
                p2Ef = p2E[:].rearrange("p h w -> p (h w)")
                p2Of = p2O[:].rearrange("p h w -> p (h w)")
                mk = p2pool.tile([P, PCR, C], F16)
                nc.vector.memset(mk[:], 1.0)
                nc.vector.memset(mk[:, :, 0:1], 0.0)
                nc.vector.memset(mk[:, :, C - 1:C], 0.0)
                mkf = mk[:].rearrange("p h w -> p (h w)")

                p2_rows = ([(62, 2), (0, 7)]
                           + [(r, min(7, 61 - r + 1)) for r in range(7, 62, 7)])
                p2_chunks = _row_chunks(p2_rows)
                conv_p(gp2t, bmt[:, 1:2], p2Ef, p2Of, 0, p2_chunks[:2])

                def wscan(rows_ap_E, rows_ap_O, mask_f):
                    # in-place W reverse cummax on an E/O row range
                    nc.vector.tensor_tensor(
                        rows_ap_O[:, :, 1:65], rows_ap_O[:, :, 1:65],
                        rows_ap_E[:, :, 2:66], AL.max,
                    )
                    flatO = rows_ap_O.rearrange("p h w -> p (h w)")
                    nc.vector.tensor_tensor_scan(
                        flatO[:, ::-1], flatO[:, ::-1], mask_f[:, ::-1],
                        0.0, AL.max, AL.mult,
                    )
                    nc.vector.tensor_tensor(
                        rows_ap_E[:], rows_ap_E[:], rows_ap_O[:], AL.max,
                    )

                for r in (62, 63, 0, 1):
                    nc.vector.memset(p2E[:, r:r + 1, C - 1:C], 0.0)
                    wscan(p2E[:, r:r + 1, :], p2O[:, r:r + 1, :],
                          mkf[:, 0:C])

                # ---- conv p1 reverse order; H-chain interleaves under it ----
                # chain step h (s rows h,h+1 = out rows h-2,h-1) is emitted as
                # soon as the reverse chunk sweep covers out row h-2, so the
                # DVE runs it ~1 chunk behind the PE instead of after conv p1.
                p1_chunks = _chunks(FLAT_P, NT)
                chain_h = [OH]

                def p1_post(q, nt):
                    lo = max((q + C - 1) // C + 2, 2)
                    for h in range(chain_h[0], lo - 1, -1):
                        nc.vector.tensor_tensor(
                            sE[:, h, :], sE[:, h, :], sE[:, h + 1, :], AL.max)
                        nc.vector.tensor_tensor(
                            sO[:, h, :], sO[:, h, :], sO[:, h + 1, :], AL.max)
                    chain_h[0] = min(chain_h[0], lo - 1)

                conv_p(gp1t, bmt[:, 0:1], sEf, sOf, 2 * C,
                       list(reversed(p1_chunks)), post_chunk=p1_post)

                # two p2 chunks before the exchange block so the PE is not
                # gated on the DVE finishing the chain tail + ct build
                conv_p(gp2t, bmt[:, 1:2], p2Ef, p2Of, 0, p2_chunks[2:4])

                # ---- pairwise exchange (E|O concat, 132-wide rows) ----
                ct = spool_s.tile([P, 8, 2 * C], F16, tag="exch")
                srcs = [
                    (sE[:, 2, :], sO[:, 2, :], m_bot),
                    (sE[:, 3, :], sO[:, 3, :], m_bot),
                    (sE[:, 2 + OH - 2, :], sO[:, 2 + OH - 2, :], m_top),
                    (sE[:, 2 + OH - 1, :], sO[:, 2 + OH - 1, :], m_top),
                    (p2E[:, 0, :], p2O[:, 0, :], m_bot),
                    (p2E[:, 1, :], p2O[:, 1, :], m_bot),
                    (p2E[:, OH - 2, :], p2O[:, OH - 2, :], m_top),
                    (p2E[:, OH - 1, :], p2O[:, OH - 1, :], m_top),
                ]
                for k, (se_, so_, m) in enumerate(srcs):
                    nc.vector.tensor_scalar_mul(ct[:, k, 0:C], se_, m)
                    nc.vector.tensor_scalar_mul(ct[:, k, C:2 * C], so_, m)
                cc_in = dpool.tile([P, 8, 2 * C], F16)
                cc_out = dpool.tile([P, 8, 2 * C], F16)
                nc.sync.dma_start(cc_in[:], ct[:])
                nc.gpsimd.collective_compute(
                    "AllReduce", AL.add, replica_groups=groups,
                    ins=[cc_in[:]], outs=[cc_out[:]],
                )
                rx = spool_s.tile([P, 8, 2 * C], F16, tag="exch2")
                nc.sync.dma_start(rx[:], cc_out[:])

                # u = R[0] + coloff (top: partner colmax; bottom: -inf)
                u = spool_s.tile([P, 2 * C], F16, tag="u")
                nc.vector.tensor_scalar_add(u[:], rx[:, 0, :], coloff)

                # ---- s-plane t-transform target ----
                with tc.tile_pool(name="st", bufs=1) as stpool:
                    st = stpool.tile([P, 4, SS, C], F16)
                    nc.vector.memset(st[:, :, :, 0:1], 0.0)
                    nc.vector.memset(st[:, :, :, C - 1:C], 0.0)
                    stf = [st[:, w4].rearrange("p h w -> p (h w)")
                           for w4 in range(4)]
                    cm = spool_s.tile([P, 2 * C], F16, tag="cm")
                    h0 = spool_s.tile([P, C], F16, tag="h0")
                    h1 = spool_s.tile([P, C], F16, tag="h1")

                    def st_xform(r0, nr):
                        # st rows r0..r0+nr from s rows (same tile rows)
                        args = [
                            (st[:, 0, r0:r0 + nr, 1:65],
                             sO[:, r0:r0 + nr, 0:64], sO[:, r0:r0 + nr, 1:65],
                             AL.subtract),
                            (st[:, 1, r0:r0 + nr, 1:65],
                             sE[:, r0:r0 + nr, 1:65], sO[:, r0:r0 + nr, 1:65],
                             AL.add),
                            (st[:, 2, r0:r0 + nr, 1:65],
                             sO[:, r0:r0 + nr, 1:65], sE[:, r0:r0 + nr, 1:65],
                             AL.subtract),
                            (st[:, 3, r0:r0 + nr, 1:65],
                             sE[:, r0:r0 + nr, 1:65], sE[:, r0:r0 + nr, 2:66],
                             AL.subtract),
                        ]
                        for i, (d, a, b_, op) in enumerate(args):
                            eng = nc.vector if i % 2 == 0 else nc.gpsimd
                            eng.tensor_tensor(d, a, b_, op)

                    def piece(pc):
                        r0 = pc * PCR
                        sr0 = 2 + r0
                        nc.vector.memset(p2E[:, r0:r0 + PCR, C - 1:C], 0.0)
                        # W reverse cummax (max ops must stay on DVE;
                        # Pool has no max TensorTensor)
                        wscan(p2E[:, r0:r0 + PCR, :], p2O[:, r0:r0 + PCR, :],
                              mkf)
                        for sp_, uc in ((sE, 0), (sO, C)):
                            nc.vector.tensor_tensor(
                                sp_[:, sr0:sr0 + PCR, :],
                                sp_[:, sr0:sr0 + PCR, :],
                                u[:, None, uc:uc + C].to_broadcast((P, PCR, C)),
                                AL.max,
                            )
                        if pc == 0:
                            nc.vector.tensor_copy(cm[:, 0:C], sE[:, 2, :])
                            nc.vector.tensor_copy(cm[:, C:2 * C], sO[:, 2, :])
                        nc.gpsimd.tensor_tensor(
                            sE[:, sr0:sr0 + PCR, :], sE[:, sr0:sr0 + PCR, :],
                            p2E[:, r0:r0 + PCR, :], AL.add)
                        nc.gpsimd.tensor_tensor(
                            sO[:, sr0:sr0 + PCR, :], sO[:, sr0:sr0 + PCR, :],
                            p2O[:, r0:r0 + PCR, :], AL.add)
                        for sp_ in (sE, sO):
                            nc.gpsimd.memset(sp_[:, sr0:sr0 + PCR, 0:1], 0.0)
                            nc.gpsimd.memset(sp_[:, sr0:sr0 + PCR, C - 1:C], 0.0)
                        if pc == 0:
                            # halo rows: above (bottom cores) s rows 0,1
                            for j in range(2):
                                for sp_, cc0 in ((sE, 0), (sO, C)):
                                    nc.vector.tensor_tensor(
                                        h0[:], rx[:, 2 + j, cc0:cc0 + C],
                                        cm[:, cc0:cc0 + C], AL.max)
                                    nc.vector.tensor_tensor(
                                        h0[:], h0[:], rx[:, 6 + j, cc0:cc0 + C],
                                        AL.add)
                                    nc.vector.tensor_scalar_mul(
                                        sp_[:, j, :], h0[:], m_bot)
                            # below (top cores) s rows 66,67
                            for j in range(2):
                                for sp_, cc0 in ((sE, 0), (sO, C)):
                                    nc.vector.tensor_tensor(
                                        h1[:], rx[:, 0 + j, cc0:cc0 + C],
                                        rx[:, 4 + j, cc0:cc0 + C], AL.add)
                                    nc.vector.tensor_scalar_mul(
                                        sp_[:, SS - 2 + j, :], h1[:], m_top)
                            for sp_ in (sE, sO):
                                for rr in (0, SS - 2):
                                    nc.vector.memset(
                                        sp_[:, rr:rr + 2, 0:1], 0.0)
                                    nc.vector.memset(
                                        sp_[:, rr:rr + 2, C - 1:C], 0.0)
                            st_xform(0, 2)
                            st_xform(SS - 2, 2)
                        st_xform(sr0, PCR)

                    # ---- conv p2 remaining chunks; each fixup piece fires
                    # as soon as its p2 rows are covered, overlapping the
                    # collective and p2's matmuls ----
                    next_pc = [0]

                    def p2_post(q, nt):
                        covered = (q + nt) // C
                        while next_pc[0] < NPC and covered >= PCR * (next_pc[0] + 1):
                            piece(next_pc[0])
                            next_pc[0] += 1

                    conv_p(gp2t, bmt[:, 1:2], p2Ef, p2Of, 0, p2_chunks[4:],
                           post_chunk=p2_post)
                    while next_pc[0] < NPC:
                        piece(next_pc[0])
                        next_pc[0] += 1

                    # gp3 into the slots gp1/gp2 free after their last chunks
                    # (emitted only now that every gp1/gp2 reader exists)
                    gp3t = [wpool.tile([P, 2, 3, 4, P], F16, name=f"gp3{t}",
                                       tag=t) for t in ("wg1", "wg2")]
                    nc.sync.dma_start(gp3t[0][:], gp3[:, 0])
                    nc.sync.dma_start(gp3t[1][:], gp3[:, 1])

                    # ---- r t-plane targets (xt slots; pads cleared) ----
                    rtt = []
                    for half in range(2):
                        rt_ = xpool.tile([P, 4, HS, C], F16, tag=f"xt{half}")
                        nc.vector.memset(rt_[:, :, :, 0:1], 0.0)
                        nc.vector.memset(rt_[:, :, :, C - 1:C], 0.0)
                        rtt.append(rt_)
                    rtf = [[rtt[s][:, w4].rearrange("p h w -> p (h w)")
                            for w4 in range(4)] for s in range(2)]

                    # ---- conv c1 (+ folded c2) -> r planes ----
                    c1_chunks = _chunks(FLAT_C, NT)
                    for half in range(2):
                        if half == 0:
                            rE = p2pool.tile([P, HS, C], F16, tag="p2E")
                            rO = p2pool.tile([P, HS, C], F16, tag="p2O")
                        else:
                            rE = spool.tile([P, HS, C], F16, tag="sE")
                            rO = spool.tile([P, HS, C], F16, tag="sO")
                        rEf = rE[:].rearrange("p h w -> p (h w)")
                        rOf = rO[:].rearrange("p h w -> p (h w)")
                        for i, (q, nt) in enumerate(c1_chunks):
                            pt = psum.tile([P, 4, NT], F32, tag="ps")
                            for w4 in range(4):
                                terms = [(gc1t[:, ky, w4, half, :], stf[w4],
                                          ky * C) for ky in range(3)]
                                if w4 == 0:
                                    terms += [(gc2t[:, s, half, :], xef[s][0], 0)
                                              for s in range(2)]
                                elif w4 == 3:
                                    terms += [(gc2t[:, s, half, :], xef[s][1], 0)
                                              for s in range(2)]
                                for j, (lhsT, rf, off) in enumerate(terms):
                                    nc.tensor.matmul(
                                        pt[:, w4, :nt], lhsT,
                                        rf[:, q + off:q + off + nt],
                                        start=(j == 0),
                                        stop=(j == len(terms) - 1),
                                    )
                            epilogue(pt, nt, bmt[:, 2 + half:3 + half],
                                     rEf[:, q:q + nt], rOf[:, q:q + nt],
                                     alt=(i % 2 == 0))
                        # mask invalid halo rows, zero pads, transform to
                        # rt right away (overlaps the other half's matmuls)
                        for rp_ in (rE, rO):
                            nc.vector.tensor_scalar_mul(
                                rp_[:, 0, :], rp_[:, 0, :], m_bot)
                            nc.vector.tensor_scalar_mul(
                                rp_[:, HS - 1, :], rp_[:, HS - 1, :], m_top)
                            nc.vector.memset(rp_[:, :, 0:1], 0.0)
                            nc.vector.memset(rp_[:, :, C - 1:C], 0.0)
                        rt_ = rtt[half]
                        for r0, nr in ((0, 17), (17, 17), (34, 16), (50, 16)):
                            args = [
                                (rt_[:, 0, r0:r0 + nr, 1:65],
                                 rO[:, r0:r0 + nr, 0:64],
                                 rO[:, r0:r0 + nr, 1:65], AL.subtract),
                                (rt_[:, 1, r0:r0 + nr, 1:65],
                                 rE[:, r0:r0 + nr, 1:65],
                                 rO[:, r0:r0 + nr, 1:65], AL.add),
                                (rt_[:, 2, r0:r0 + nr, 1:65],
                                 rO[:, r0:r0 + nr, 1:65],
                                 rE[:, r0:r0 + nr, 1:65], AL.subtract),
                                (rt_[:, 3, r0:r0 + nr, 1:65],
                                 rE[:, r0:r0 + nr, 1:65],
                                 rE[:, r0:r0 + nr, 2:66], AL.subtract),
                            ]
                            for i, (d, a, b_, op) in enumerate(args):
                                eng = nc.vector if i % 2 == 0 else nc.gpsimd
                                eng.tensor_tensor(d, a, b_, op)

                    # ---- conv p3 -> interleaved fp16 staging -> out ----
                    p3_rows = [(r, 6) for r in range(0, 60, 6)] + [(60, 4)]
                    for half in range(2):
                        stg = xpool.tile([P, OH, W], F16, tag=f"xe{half}")
                        for i, (r0, nr) in enumerate(p3_rows):
                            q, nt = r0 * C, nr * C
                            pt = psum.tile([P, 4, NT], F32, tag="ps")
                            for w4 in range(4):
                                terms = [(gp3t[s][:, half, ky, w4, :],
                                          rtf[s][w4], ky * C)
                                         for s in range(2) for ky in range(3)]
                                for j, (lhsT, rf, off) in enumerate(terms):
                                    nc.tensor.matmul(
                                        pt[:, w4, :nt], lhsT,
                                        rf[:, q + off:q + off + nt],
                                        start=(j == 0),
                                        stop=(j == len(terms) - 1),
                                    )
                            ta = epool.tile([P, 6, C], F32, tag="ta")
                            tb = epool.tile([P, 6, C], F32, tag="tb")
                            taf = ta[:].rearrange("p h w -> p (h w)")
                            tbf = tb[:].rearrange("p h w -> p (h w)")
                            bias = bmt[:, 4 + half:5 + half]
                            nc.scalar.activation(taf[:, :nt], pt[:, 1, :nt],
                                                 AF.Copy)
                            nc.vector.tensor_tensor(
                                tbf[:, :nt], taf[:, :nt], pt[:, 0, :nt], AL.add)
                            nc.vector.tensor_tensor(
                                tbf[:, :nt], tbf[:, :nt], pt[:, 2, :nt], AL.add)
                            nc.scalar.activation(
                                stg[:, r0:r0 + nr, 0:W:2],
                                tb[:, :nr, 1:65], AF.Relu, bias=bias)
                            nc.vector.tensor_tensor(
                                taf[:, :nt], taf[:, :nt], pt[:, 2, :nt],
                                AL.subtract)
                            nc.vector.tensor_tensor(
                                taf[:, :nt], taf[:, :nt], pt[:, 3, :nt],
                                AL.subtract)
                            nc.scalar.activation(
                                stg[:, r0:r0 + nr, 1:W:2],
                                ta[:, :nr, 1:65], AF.Relu, bias=bias)
                            # per-chunk output DMA: rows ship the moment
                            # their relu lands, shrinking the kernel tail
                            nc.sync.dma_start(
                                o[half, :, r0:r0 + nr, :],
                                stg[:, r0:r0 + nr, :])

    _fix_multiwaits(nc)
    return nc


_NC = None


def _get_nc():
    global _NC
    if _NC is None:
        _NC = build_nc()
    return _NC


def _fold_bn(w, g, b, m, v):
    s = (np.asarray(g) / np.sqrt(np.asarray(v) + EPS)).astype(np.float32)
    t = (np.asarray(b) - np.asarray(m) * s).astype(np.float32)
    return np.asarray(w, np.float32) * s[:, None, None, None], t


def _wino_w(w):
    # w [O, I, 3, 3] -> G [4, 3ky, I, O]
    g0, g1, g2 = w[..., 0], w[..., 1], w[..., 2]
    G = np.stack([g0, (g0 + g1 + g2) * 0.5, (g0 - g1 + g2) * 0.5, g2])
    return G.transpose(0, 3, 2, 1).astype(np.float16)


def kernel(**inputs):
    x = np.asarray(inputs["x"], np.float32)

    w_p1, t_p1 = _fold_bn(inputs["w_p1"], inputs["g_p1"], inputs["b_p1"],
                          inputs["m_p1"], inputs["v_p1"])
    w_p2, t_p2 = _fold_bn(inputs["w_p2"], inputs["g_p2"], inputs["b_p2"],
                          inputs["m_p2"], inputs["v_p2"])
    w_c1, t_c1 = _fold_bn(inputs["w_c1"], inputs["g_c1"], inputs["b_c1"],
                          inputs["m_c1"], inputs["v_c1"])
    w_c2, t_c2 = _fold_bn(inputs["w_c2"], inputs["g_c2"], inputs["b_c2"],
                          inputs["m_c2"], inputs["v_c2"])
    w_p3, t_p3 = _fold_bn(inputs["w_p3"], inputs["g_p3"], inputs["b_p3"],
                          inputs["m_p3"], inputs["v_p3"])

    Gp1 = _wino_w(w_p1)  # [4,3,256,128]
    Gp2 = _wino_w(w_p2)
    Gc1 = _wino_w(w_c1)  # [4,3,128,256]
    Gp3 = _wino_w(w_p3)  # [4,3,256,256]

    gp1a = np.ascontiguousarray(
        Gp1.reshape(4, 3, 2, P, P).transpose(3, 2, 1, 0, 4))
    gp2a = np.ascontiguousarray(
        Gp2.reshape(4, 3, 2, P, P).transpose(3, 2, 1, 0, 4))
    gc1a = np.ascontiguousarray(
        Gc1.reshape(4, 3, P, 2, P).transpose(2, 1, 0, 3, 4))
    gp3a = np.ascontiguousarray(
        Gp3.reshape(4, 3, 2, P, 2, P).transpose(3, 2, 4, 1, 0, 5))
    gc2a = np.ascontiguousarray(
        w_c2[:, :, 0, 0].reshape(2, P, 2, P).transpose(3, 2, 0, 1)
    ).astype(np.float16)

    bias = np.zeros((P, 6), np.float32)
    bias[:, 0] = t_p1
    bias[:, 1] = t_p2
    bc = t_c1 + t_c2
    bias[:, 2] = bc[:P]
    bias[:, 3] = bc[P:]
    bias[:, 4] = t_p3[:P]
    bias[:, 5] = t_p3[P:]

    # x slabs per core-half with H halo, as fp16 E/O planes + t-planes
    x16 = x.astype(np.float16).astype(np.float32)
    xr = x16.reshape(B, 2, P, H, W)
    pad = np.zeros((B, 2, 2, P, HS, W), np.float32)  # [b, half, slab, p, h, w]
    pad[:, 0, :, :, 1:HS, :] = xr[:, :, :, 0:65, :]
    pad[:, 1, :, :, 0:HS - 1, :] = xr[:, :, :, 63:128, :]
    xE = np.zeros((B, 2, 2, P, HS, C), np.float32)
    xO = np.zeros_like(xE)
    xE[..., 1:65] = pad[..., 0::2]
    xO[..., 1:65] = pad[..., 1::2]
    t4 = np.zeros((B, 2, 2, P, 4, HS, C), np.float32)
    t4[..., 0, :, 1:65] = xO[..., 0:64] - xO[..., 1:65]
    t4[..., 1, :, 1:65] = xE[..., 1:65] + xO[..., 1:65]
    t4[..., 2, :, 1:65] = xO[..., 1:65] - xE[..., 1:65]
    t4[..., 3, :, 1:65] = xE[..., 1:65] - xE[..., 2:66]
    t4 = t4.astype(np.float16)
    xeo = np.stack([xE, -xO], axis=4).astype(np.float16)  # [b,half,slab,p,2,h,c]

    wmaps = {"gp1": gp1a, "gp2": gp2a, "gc1": gc1a, "gc2": gc2a, "gp3": gp3a}
    in_maps = []
    for b in range(B):
        for half in range(2):
            bmv = np.zeros((P, 12), np.float32)
            bmv[:, 0:6] = bias
            if half == 0:  # top
                bmv[:, 6] = 1.0
                bmv[:, 8] = 0.0
            else:  # bottom
                bmv[:, 7] = 1.0
                bmv[:, 8] = -1e30
            in_maps.append({
                "xt": t4[b, half], "xe": xeo[b, half], "bm": bmv, **wmaps,
            })

    global _last_in_maps
    _last_in_maps = in_maps

    nc = _get_nc()
    res = run_bass_kernel_spmd(nc, in_maps, list(range(8)))

    out = np.empty((B, CIN, H, W), np.float32)
    for b in range(B):
        out[b, :, 0:OH] = res.results[2 * b]["o"].reshape(CIN, OH, W)
        out[b, :, OH:H] = res.results[2 * b + 1]["o"].reshape(CIN, OH, W)
    return out


if __name__ == "__main__":
    import reference

    inp = {k: np.asarray(v) for k, v in reference.setup_inputs().items()}
    exp = np.asarray(reference.reference(**inp))
    got = kernel(**inp)
    err = np.abs(got - exp)
    rel = err.max() / max(np.abs(exp).max(), 1e-6)
    print("abs err max:", err.max(), "rel (vs absmax):", rel)
